# revision 1
# baseline (speedup 1.0000x reference)
"""Trainium2 Bass kernel for the DistancePositionOperator.

Reference computation (B=2, L=1024, D=128):
    delta[b,i,j,:] = X[b,i,:] - X[b,j,:]
    alpha[i,j]     = 1 / (1 + |i-j|)            (zero on the diagonal)
    d[b,i,j]       = sum_d |delta|              (pairwise L1 distance)
    C[b,i,j]       = alpha[i,j] / (1 + d[b,i,j])
    O[b,i,:]       = sum_j C[b,i,j] * delta[b,i,j,:]
                   = rowsum(C)[b,i] * X[b,i,:] - (C @ X)[b,i,:]

d and C are symmetric in (i,j), so only one of each 128x128 block pair
is computed: with L split into 8 strip-blocks that is 36 blocks per
batch, 72 total -> 9 per core.  Core q (batch q//4, q%4 -> rotation)
computes blocks (I, (I+K) mod 8) for K in 0..3 at I in {q, q+4} plus the
(q, q+4) anti-diagonal block.  Host-side each core's inputs are rotated
by 128*q tokens so every core runs the IDENTICAL program: strip 0
against key span [0,640) and strip 4 against [512,1024), both
contiguous.  The host un-rotates and sums the per-core partial outputs.

Per query row i the elementwise engines (ACT: Abs activation with
per-partition bias; DVE: custom |x - c| uop chain) emit
Abs_i[d, jspan] in bf16.  The PE reduces over d by using Abs_i as
matmul *weights* against a ones vector, landing dT[j, i] directly in
PSUM in the transposed layout needed downstream.  C^T = alpha^T/(1+dT)
then drives output matmuls (C^T as weights, [X | 1] as moving) which
produce C@X and rowsum(C) in one pass; the mirror contribution uses the
PE-transposed C block.
"""

import numpy as np
import ml_dtypes

B, L, D = 2, 1024, 128
NBLK = L // 128                      # 8 strip blocks per batch
N_CORES = 8
# per-strip i -> engine split (ACT, DVE, Pool): balances engine busy time;
# Pool uses a 2-instruction relu/min path with +-ones PE accumulation.
STRIP_SPLIT = {0: (58, 51, 19), 4: (60, 49, 19)}


def _engine_pattern(n_act, n_dve, n_pool):
    counts = [n_act, n_dve, n_pool]
    acc = [0.0, 0.0, 0.0]
    pat = []
    for _ in range(sum(counts)):
        for e in range(3):
            acc[e] += counts[e]
        e = max(range(3), key=lambda k: acc[k])
        acc[e] -= sum(counts)
        pat.append("AVP"[e])
    return pat

# program-relative schedule (identical on every core):
#   (query strip, [key blocks])
SCHED = [(0, [0, 1, 2, 3, 4]), (4, [4, 5, 6, 7])]
NBLOCKS = 9

_COMPILED = None


def _register_abs_diff():
    import concourse.dve_ops as dve_ops
    from concourse.dve_spec import Spec, Src0, C0, maxx, lower as dve_lower, _has_src1
    from concourse.dve_uop import DveOpSpec

    name = "ABS_DIFF_ANT_X"
    for op in dve_ops.OPS:
        if op.name == name:
            return op
    spec = Spec(
        body=maxx(Src0 - C0, C0 - Src0),
        reference=lambda in0, in1, s0, s1, imm2: np.abs(in0.astype(np.float32) - s0),
    )
    row = max(dve_ops._SUB_OPCODE_FOR_NAME.values()) + 1
    shas = {}
    for ver in ("v3", "v4"):
        s = DveOpSpec(name=name, opcode=row, uops=dve_lower(spec, ver=ver),
                      rd1_en=_has_src1(spec))
        shas[ver] = s.sha(ver)
    op = dve_ops.DveOp(name, spec, subdim=False, uops_sha=shas)
    dve_ops.OPS.append(op)
    dve_ops._SUB_OPCODE_FOR_NAME[name] = row
    dve_ops.CUSTOM_DVE_SPECS[name] = spec
    return op


def _build(iters=1):
    """Build + compile the (core-uniform) Bass program."""
    import concourse.bacc as bacc
    import concourse.tile as tile
    import concourse.mybir as mybir
    from concourse.masks import make_identity

    ABS_DIFF = _register_abs_diff()
    F32, BF16 = mybir.dt.float32, mybir.dt.bfloat16
    AF = mybir.ActivationFunctionType
    ALU = mybir.AluOpType

    nc = bacc.Bacc("TRN2", target_bir_lowering=False, debug=False,
                   num_devices=N_CORES)
    xt16_ap = nc.dram_tensor("xt16", [D, L], BF16, kind="ExternalInput").ap()
    xt32_ap = nc.dram_tensor("xt32b", [D, 256], F32, kind="ExternalInput").ap()
    xaug_ap = nc.dram_tensor("xaug", [NBLK, 128, D + 1], F32,
                             kind="ExternalInput").ap()
    alpha_ap = nc.dram_tensor("alphat", [NBLOCKS, 128, 128], F32,
                              kind="ExternalInput").ap()
    pout_ap = nc.dram_tensor("pout", [NBLK, 128, D], F32,
                             kind="ExternalOutput").ap()

    with tile.TileContext(nc) as tc:
        with tc.tile_pool(name="consts", bufs=1) as consts, \
             tc.tile_pool(name="abs", bufs=14) as abs_pool, \
             tc.tile_pool(name="work", bufs=3) as work, \
             tc.tile_pool(name="dtps", bufs=5, space="PSUM") as dtps, \
             tc.tile_pool(name="ops", bufs=2, space="PSUM") as ops_ps, \
             tc.tile_pool(name="tps", bufs=1, space="PSUM") as tps:

            xt16 = consts.tile([D, L], BF16, tag="xt16")
            xt32 = consts.tile([D, 256], F32, tag="xt32")
            nc.sync.dma_start(xt32[:, 0:128], xt32_ap[:, 0:128])
            nc.sync.dma_start(xt16[:, 0:640], xt16_ap[:, 0:640])
            nc.sync.dma_start(xt32[:, 128:256], xt32_ap[:, 128:256])
            nc.sync.dma_start(xt16[:, 640:L], xt16_ap[:, 640:L])
            xaug = consts.tile([128, NBLK * (D + 1)], F32, tag="xaug")
            for s in range(NBLK):
                nc.sync.dma_start(xaug[:, s * (D + 1):(s + 1) * (D + 1)],
                                  xaug_ap[s])
            alpha = consts.tile([128, NBLOCKS * 128], F32, tag="alpha")
            for k in range(NBLOCKS):
                nc.sync.dma_start(alpha[:, k * 128:(k + 1) * 128], alpha_ap[k])
            ones16 = consts.tile([D, 1], BF16, tag="ones")
            nc.vector.memset(ones16[:], 1.0)
            nones16 = consts.tile([D, 1], BF16, tag="nones")
            nc.vector.memset(nones16[:], -1.0)
            ident = consts.tile([128, 128], F32, tag="ident")
            make_identity(nc, ident[:])

            import contextlib
            loop_cm = (tc.For_i(0, iters, 1) if iters > 1
                       else contextlib.nullcontext())
            with loop_cm:
                _kernel_body(nc, tc, mybir, ABS_DIFF, xt16, xt32, xaug, alpha,
                             ones16, nones16, ident, consts, abs_pool, work,
                             dtps, ops_ps, tps, pout_ap)

    nc.compile()
    return nc


def _kernel_body(nc, tc, mybir, ABS_DIFF, xt16, xt32, xaug, alpha, ones16,
                 nones16, ident, consts, abs_pool, work, dtps, ops_ps, tps,
                 pout_ap):
    F32, BF16 = mybir.dt.float32, mybir.dt.bfloat16
    AF = mybir.ActivationFunctionType
    ALU = mybir.AluOpType
    if True:
        if True:
            oacc = []
            for s in range(NBLK):
                t = consts.tile([128, D + 1], F32, tag=f"oacc{s}")
                nc.gpsimd.memset(t[:], 0.0)
                oacc.append(t)

            def xaug_blk(s):
                return xaug[:, s * (D + 1):(s + 1) * (D + 1)]

            blk = 0
            for R, jblocks in SCHED:
                nb = len(jblocks)
                jlo = jblocks[0] * 128
                w = nb * 128
                dts = [dtps.tile([128, 128], F32, tag="dt", name=f"dt{R}_{k}")
                       for k in range(nb)]
                pat = _engine_pattern(*STRIP_SPLIT[R])
                boff = 0 if R == 0 else 128
                for i in range(128):
                    gi = boff + i
                    eng = pat[i]
                    if eng == "P":
                        pp = abs_pool.tile([D, w], BF16, tag="ab", name=f"pp{R}_{i}")
                        nc.gpsimd.tensor_scalar(
                            pp[:], xt16[:, jlo:jlo + w], xt32[:, gi:gi + 1],
                            0.0, ALU.subtract, ALU.max)
                        mm = abs_pool.tile([D, w], BF16, tag="ab", name=f"mm{R}_{i}")
                        nc.gpsimd.tensor_scalar(
                            mm[:], xt16[:, jlo:jlo + w], xt32[:, gi:gi + 1],
                            0.0, ALU.subtract, ALU.min)
                        for k in range(nb):
                            nc.tensor.matmul(
                                dts[k][:, i:i + 1],
                                lhsT=pp[:, k * 128:(k + 1) * 128],
                                rhs=ones16[:], start=True, stop=False)
                            nc.tensor.matmul(
                                dts[k][:, i:i + 1],
                                lhsT=mm[:, k * 128:(k + 1) * 128],
                                rhs=nones16[:], start=False, stop=True)
                        continue
                    ab = abs_pool.tile([D, w], BF16, tag="ab", name=f"ab{R}_{i}")
                    if eng == "A":
                        nc.scalar.activation(
                            ab[:], xt16[:, jlo:jlo + w], AF.Abs,
                            bias=xt32[:, gi:gi + 1], scale=-1.0)
                    else:
                        nc.vector._custom_dve(
                            ABS_DIFF, out=ab[:], in0=xt16[:, jlo:jlo + w],
                            s0=xt32[:, gi:gi + 1])
                    for k in range(nb):
                        nc.tensor.matmul(
                            dts[k][:, i:i + 1],
                            lhsT=ab[:, k * 128:(k + 1) * 128],
                            rhs=ones16[:], start=True, stop=True)
                for k, J in enumerate(jblocks):
                    # u = 1 + dT   (PSUM -> SBUF, frees the dt bank)
                    u = work.tile([128, 128], F32, tag="u")
                    nc.vector.tensor_scalar_add(u[:], dts[k][:], 1.0)
                    r = work.tile([128, 128], F32, tag="r")
                    nc.vector.reciprocal_approx_fast(r[:], u[:])
                    ct = work.tile([128, 128], F32, tag="ct")
                    nc.gpsimd.tensor_tensor(
                        ct[:], r[:], alpha[:, blk * 128:(blk + 1) * 128],
                        ALU.mult)
                    # O_R partial: [C@X | rowsum] over keys in block J
                    po = ops_ps.tile([128, D + 1], F32, tag="po")
                    nc.tensor.matmul(po[:], lhsT=ct[:], rhs=xaug_blk(J),
                                     start=True, stop=True)
                    nc.vector.tensor_add(oacc[R][:], oacc[R][:], po[:])
                    if J != R:
                        pt = tps.tile([128, 128], F32, tag="pt")
                        nc.tensor.transpose(pt[:], ct[:], ident[:])
                        ctT = work.tile([128, 128], F32, tag="ctT")
                        nc.scalar.copy(ctT[:], pt[:])
                        po2 = ops_ps.tile([128, D + 1], F32, tag="po")
                        nc.tensor.matmul(po2[:], lhsT=ctT[:],
                                         rhs=xaug_blk(R),
                                         start=True, stop=True)
                        nc.vector.tensor_add(oacc[J][:], oacc[J][:], po2[:])
                    blk += 1

            # O_s = rowsum * X_s - (C@X)_s  =  X_s*oacc[:,D] - oacc[:,:D]
            for s in range(NBLK):
                o = work.tile([128, D], F32, tag="fin")
                nc.vector.scalar_tensor_tensor(
                    o[:], xaug_blk(s)[:, 0:D], oacc[s][:, D:D + 1],
                    oacc[s][:, 0:D], ALU.mult, ALU.subtract)
                nc.sync.dma_start(pout_ap[s], o[:])


_ALPHA_CACHE = {}


def _core_alpha(q):
    if q in _ALPHA_CACHE:
        return _ALPHA_CACHE[q]
    idx = np.arange(L, dtype=np.float64)
    rot = 128 * q
    real = (idx + rot) % L
    al = np.empty((NBLOCKS, 128, 128), dtype=np.float32)
    k = 0
    for R, jblocks in SCHED:
        ti = real[R * 128:(R + 1) * 128]
        for J in jblocks:
            tj = real[J * 128:(J + 1) * 128]
            dist = np.abs(tj[:, None] - ti[None, :])
            a = 1.0 / (1.0 + dist)
            a[dist == 0] = 0.0
            al[k] = a.astype(np.float32)
            k += 1
    _ALPHA_CACHE[q] = al
    return al


def _prep_host(X):
    """Per-core rotated input dicts. X: [B, L, D] float32."""
    in_maps = []
    for c in range(N_CORES):
        b, q = c // 4, c % 4
        rot = 128 * q
        Xr = np.roll(X[b], -rot, axis=0)          # program token t = real t+rot
        xtT = np.ascontiguousarray(Xr.T)          # [D, L]
        xt16 = xtT.astype(ml_dtypes.bfloat16)
        xt32b = np.ascontiguousarray(
            np.concatenate([xtT[:, 0:128], xtT[:, 512:640]], axis=1))
        xaug = np.concatenate(
            [Xr, np.ones((L, 1), dtype=np.float32)], axis=1)
        xaug = np.ascontiguousarray(xaug.reshape(NBLK, 128, D + 1))
        in_maps.append({"xt16": xt16, "xt32b": xt32b, "xaug": xaug,
                        "alphat": _core_alpha(q)})
    return in_maps


def _get_compiled():
    global _COMPILED
    if _COMPILED is None:
        _COMPILED = _build()
    return _COMPILED


def kernel(X, _trace=False, _trace_kwargs=None):
    """X: np.ndarray [2, 1024, 128] float32 -> O [2, 1024, 128] float32."""
    from concourse.bass_utils import run_bass_kernel_spmd

    X = np.asarray(X, dtype=np.float32)
    assert X.shape == (B, L, D)
    nc = _get_compiled()
    in_maps = _prep_host(X)
    res = run_bass_kernel_spmd(nc, in_maps, list(range(N_CORES)),
                               trace=_trace, **(_trace_kwargs or {}))
    O = np.zeros((B, L, D), dtype=np.float32)
    for c in range(N_CORES):
        b, q = c // 4, c % 4
        part = res.results[c]["pout"].reshape(L, D)
        O[b] += np.roll(part, 128 * q, axis=0)    # un-rotate
    if _trace:
        return O, res
    return O


if __name__ == "__main__":
    rng = np.random.default_rng(0)
    X = rng.standard_normal((B, L, D), dtype=np.float32)
    O = kernel(X)
    print("ok", O.shape, float(np.abs(O).max()))



# revision 12
# speedup vs baseline: 5.2181x; 5.2181x over previous
"""Trainium2 Bass kernel for the DistancePositionOperator.

Reference computation (B=2, L=1024, D=128):
    delta[b,i,j,:] = X[b,i,:] - X[b,j,:]
    alpha[i,j]     = 1 / (1 + |i-j|)            (zero on the diagonal)
    d[b,i,j]       = sum_d |delta|              (pairwise L1 distance)
    C[b,i,j]       = alpha[i,j] / (1 + d[b,i,j])
    O[b,i,:]       = sum_j C[b,i,j] * delta[b,i,j,:]
                   = rowsum(C)[b,i] * X[b,i,:] - (C @ X)[b,i,:]

d and C are symmetric in (i,j), so only one of each 128x128 block pair
is computed: with L split into 8 strip-blocks that is 36 blocks per
batch, 72 total -> 9 per core.  Core q (batch q//4, q%4 -> rotation)
computes blocks (I, (I+K) mod 8) for K in 0..3 at I in {q, q+4} plus the
(q, q+4) anti-diagonal block.  Host-side each core's inputs are rotated
by 128*q tokens so every core runs the IDENTICAL program: strip 0
against key span [0,640) and strip 4 against [512,1024), both
contiguous.  The host un-rotates and sums the per-core partial outputs.

Per query row i the elementwise engines (ACT: Abs activation with
per-partition bias; DVE: custom |x - c| uop chain) emit
Abs_i[d, jspan] in bf16.  The PE reduces over d by using Abs_i as
matmul *weights* against a ones vector, landing dT[j, i] directly in
PSUM in the transposed layout needed downstream.  The ACT engine turns
dT into rT = 1/(1+dT) in one Reciprocal-activation pass (Abs and
Reciprocal share the reciprocal_and_small table set), gpsimd applies
alpha^T, and the PE then drives the output matmuls (C^T as weights,
[X | 1] bf16 as moving), accumulating the strip streams directly in
PSUM; mirror-block outputs are finalized and DMA'd out as soon as
their single contribution lands.
"""

import numpy as np
import ml_dtypes

B, L, D = 2, 1024, 128
NBLK = L // 128                      # 8 strip blocks per batch
N_CORES = 8
# per-strip i -> engine split (ACT, DVE): balances engine busy time.
STRIP_SPLIT = {0: (66, 62), 4: (70, 58)}


def _engine_pattern(n_act, n_dve):
    counts = [n_act, n_dve]
    acc = [0.0, 0.0]
    pat = []
    for _ in range(sum(counts)):
        for e in range(2):
            acc[e] += counts[e]
        e = max(range(2), key=lambda k: acc[k])
        acc[e] -= sum(counts)
        pat.append("AV"[e])
    return pat

# program-relative schedule (identical on every core):
#   (query strip, [key blocks])
SCHED = [(0, [0, 1, 2, 3, 4]), (4, [4, 5, 6, 7])]
NBLOCKS = 9

_COMPILED = None


def _register_abs_diff():
    import concourse.dve_ops as dve_ops
    from concourse.dve_spec import Spec, Src0, C0, maxx, lower as dve_lower, _has_src1
    from concourse.dve_uop import DveOpSpec

    name = "ABS_DIFF_ANT_X"
    for op in dve_ops.OPS:
        if op.name == name:
            return op
    spec = Spec(
        body=maxx(Src0 - C0, C0 - Src0),
        reference=lambda in0, in1, s0, s1, imm2: np.abs(in0.astype(np.float32) - s0),
    )
    row = max(dve_ops._SUB_OPCODE_FOR_NAME.values()) + 1
    shas = {}
    for ver in ("v3", "v4"):
        s = DveOpSpec(name=name, opcode=row, uops=dve_lower(spec, ver=ver),
                      rd1_en=_has_src1(spec))
        shas[ver] = s.sha(ver)
    op = dve_ops.DveOp(name, spec, subdim=False, uops_sha=shas)
    dve_ops.OPS.append(op)
    dve_ops._SUB_OPCODE_FOR_NAME[name] = row
    dve_ops.CUSTOM_DVE_SPECS[name] = spec
    return op


def _build(iters=1):
    """Build + compile the (core-uniform) Bass program."""
    import concourse.bacc as bacc
    import concourse.tile as tile
    import concourse.mybir as mybir
    from concourse.masks import make_identity

    ABS_DIFF = _register_abs_diff()
    F32, BF16 = mybir.dt.float32, mybir.dt.bfloat16

    nc = bacc.Bacc("TRN2", target_bir_lowering=False, debug=False,
                   num_devices=N_CORES)
    xt16_ap = nc.dram_tensor("xt16", [D, L], BF16, kind="ExternalInput").ap()
    xt32_ap = nc.dram_tensor("xt32b", [D, 256], F32, kind="ExternalInput").ap()
    xaug_ap = nc.dram_tensor("xaug", [NBLK, 128, D + 1], F32,
                             kind="ExternalInput").ap()
    xaug16_ap = nc.dram_tensor("xaug16", [NBLK, 128, D + 1], BF16,
                               kind="ExternalInput").ap()
    alpha_ap = nc.dram_tensor("alphat", [NBLOCKS, 128, 128], F32,
                              kind="ExternalInput").ap()
    pout_ap = nc.dram_tensor("pout", [NBLK, 128, D], F32,
                             kind="ExternalOutput").ap()

    with tile.TileContext(nc) as tc:
        with tc.tile_pool(name="consts", bufs=1) as consts, \
             tc.tile_pool(name="abs", bufs=14) as abs_pool, \
             tc.tile_pool(name="work", bufs=4) as work, \
             tc.tile_pool(name="dtps", bufs=3, space="PSUM") as dtps, \
             tc.tile_pool(name="oacc", bufs=1, space="PSUM") as oacc_ps, \
             tc.tile_pool(name="po2", bufs=1, space="PSUM") as po2_ps, \
             tc.tile_pool(name="tps", bufs=1, space="PSUM") as tps:

            xt16 = consts.tile([D, L], BF16, tag="xt16")
            xt32 = consts.tile([D, 256], F32, tag="xt32")
            nc.sync.dma_start(xt32[:, 0:128], xt32_ap[:, 0:128])
            nc.sync.dma_start(xt16[:, 0:640], xt16_ap[:, 0:640])
            nc.sync.dma_start(xt32[:, 128:256], xt32_ap[:, 128:256])
            nc.sync.dma_start(xt16[:, 640:L], xt16_ap[:, 640:L])
            xaug = consts.tile([128, NBLK * (D + 1)], F32, tag="xaug")
            xaug16 = consts.tile([128, NBLK * (D + 1)], BF16, tag="xaug16")
            for s in range(NBLK):
                nc.sync.dma_start(xaug[:, s * (D + 1):(s + 1) * (D + 1)],
                                  xaug_ap[s])
                nc.sync.dma_start(xaug16[:, s * (D + 1):(s + 1) * (D + 1)],
                                  xaug16_ap[s])
            alpha = consts.tile([128, NBLOCKS * 128], F32, tag="alpha")
            for k in range(NBLOCKS):
                nc.sync.dma_start(alpha[:, k * 128:(k + 1) * 128], alpha_ap[k])
            ones16 = consts.tile([D, 1], BF16, tag="ones")
            nc.vector.memset(ones16[:], 1.0)
            ident = consts.tile([128, 128], BF16, tag="ident")
            make_identity(nc, ident[:])

            import contextlib
            loop_cm = (tc.For_i(0, iters, 1) if iters > 1
                       else contextlib.nullcontext())
            with loop_cm:
                _kernel_body(nc, tc, mybir, ABS_DIFF, xt16, xt32, xaug, xaug16,
                             alpha, ones16, ident, consts, abs_pool, work,
                             dtps, oacc_ps, po2_ps, tps, pout_ap)

    nc.compile()
    return nc


def _kernel_body(nc, tc, mybir, ABS_DIFF, xt16, xt32, xaug, xaug16, alpha,
                 ones16, ident, consts, abs_pool, work, dtps, oacc_ps, po2_ps,
                 tps, pout_ap):
    F32, BF16 = mybir.dt.float32, mybir.dt.bfloat16
    AF = mybir.ActivationFunctionType
    ALU = mybir.AluOpType

    def xaug_blk(s):
        return xaug[:, s * (D + 1):(s + 1) * (D + 1)]

    def xaug16_blk(s):
        return xaug16[:, s * (D + 1):(s + 1) * (D + 1)]

    # Oacc_R streams: direct contributions of strip R accumulate in PSUM.
    oacc = {R: oacc_ps.tile([128, D + 1], F32, tag=f"oacc{R}",
                            name=f"oacc{R}")
            for R, _ in SCHED}

    def final_out(J, src):
        # O_J = X_J * rowsum - C@X   (src: PSUM [128, D+1]).  gpsimd
        # cannot read PSUM (and Pool has no scalar_tensor_tensor), so ACT
        # stages src into SBUF and gpsimd runs mul + sub on SBUF.
        pof = work.tile([128, D + 1], F32, tag="pof")
        nc.scalar.copy(pof[:], src[:])
        tmp = work.tile([128, D], F32, tag="ftmp")
        nc.gpsimd.tensor_scalar_mul(tmp[:], xaug_blk(J)[:, 0:D],
                                    pof[:, D:D + 1])
        o = work.tile([128, D], F32, tag="fin")
        nc.gpsimd.tensor_tensor(o[:], tmp[:], pof[:, 0:D], ALU.subtract)
        nc.sync.dma_start(pout_ap[J], o[:])

    blk = 0
    for si, (R, jblocks) in enumerate(SCHED):
        nb = len(jblocks)
        jlo = jblocks[0] * 128
        w = nb * 128
        # dt^T for this strip, packed 4 key-blocks per PSUM bank:
        # strip0: bank A holds J=0..3, bank B holds J=4 (first 128 cols);
        # strip4: one bank holds J=4..7.
        dtbanks = []
        nfull = nb // 4
        for t in range(nfull):
            dtbanks.append(dtps.tile([128, 512], F32, tag="dtb",
                                     name=f"dtb{R}_{t}"))
        if nb % 4:
            dtbanks.append(dtps.tile([128, (nb % 4) * 128], F32, tag="dtb",
                                     name=f"dtb{R}_r"))
        # Pre-fill with 1.0: the row matmuls accumulate (start=False) on
        # top, landing 1 + dT directly so the reciprocal needs no add.
        # (gpsimd cannot touch PSUM, so this is a DVE memset)
        for bank in dtbanks:
            nc.vector.memset(bank[:], 1.0)

        def dt_slice(k, i0=None, i1=None):
            bank = dtbanks[k // 4]
            o = (k % 4) * 128
            if i0 is None:
                return bank[:, o:o + 128]
            return bank[:, o + i0:o + i1]

        pat = _engine_pattern(*STRIP_SPLIT[R])
        boff = 0 if R == 0 else 128
        for i in range(128):
            gi = boff + i
            ab = abs_pool.tile([D, w], BF16, tag="ab", name=f"ab{R}_{i}")
            if pat[i] == "A":
                nc.scalar.activation(
                    ab[:], xt16[:, jlo:jlo + w], AF.Abs,
                    bias=xt32[:, gi:gi + 1], scale=-1.0)
            else:
                nc.vector._custom_dve(
                    ABS_DIFF, out=ab[:], in0=xt16[:, jlo:jlo + w],
                    s0=xt32[:, gi:gi + 1])
            for k in range(nb):
                nc.tensor.matmul(
                    dt_slice(k, i, i + 1),
                    lhsT=ab[:, k * 128:(k + 1) * 128],
                    rhs=ones16[:], start=False, stop=True,
                    skip_group_check=True)

        for k, J in enumerate(jblocks):
            # rT = 1/(1 + dT): PSUM already holds 1 + dT
            r = work.tile([128, 128], F32, tag="r")
            nc.vector.reciprocal_approx_fast(r[:], dt_slice(k))
            ct = work.tile([128, 128], BF16, tag="ct")
            nc.gpsimd.tensor_tensor(
                ct[:], r[:], alpha[:, blk * 128:(blk + 1) * 128], ALU.mult)
            # direct: O_R += C_RJ @ [X_J | 1]   (lhsT = ct = C^T as weights)
            nc.tensor.matmul(oacc[R][:], lhsT=ct[:], rhs=xaug16_blk(J),
                             start=(k == 0 and R == 0), stop=(k == nb - 1),
                             skip_group_check=True)
            if J != R:
                # mirror: O_J += C_RJ^T @ [X_R | 1]  (lhsT = transpose(ct))
                pt = tps.tile([128, 128], BF16, tag="pt")
                nc.tensor.transpose(pt[:], ct[:], ident[:])
                ctT = work.tile([128, 128], BF16, tag="ctT")
                nc.scalar.copy(ctT[:], pt[:])
                if J == 4:
                    # lands in strip4's accumulation stream (first entry)
                    nc.tensor.matmul(oacc[4][:], lhsT=ctT[:],
                                     rhs=xaug16_blk(R), start=True,
                                     stop=False, skip_group_check=True)
                else:
                    po2 = po2_ps.tile([128, D + 1], F32, tag="po2")
                    nc.tensor.matmul(po2[:], lhsT=ctT[:], rhs=xaug16_blk(R),
                                     start=True, stop=True)
                    final_out(J, po2)
            blk += 1
        # strip R's own output once its accumulation stream is closed
        final_out(R, oacc[R])


_ALPHA_CACHE = {}


def _core_alpha(q):
    if q in _ALPHA_CACHE:
        return _ALPHA_CACHE[q]
    idx = np.arange(L, dtype=np.float64)
    rot = 128 * q
    real = (idx + rot) % L
    al = np.empty((NBLOCKS, 128, 128), dtype=np.float32)
    k = 0
    for R, jblocks in SCHED:
        ti = real[R * 128:(R + 1) * 128]
        for J in jblocks:
            tj = real[J * 128:(J + 1) * 128]
            dist = np.abs(tj[:, None] - ti[None, :])
            a = 1.0 / (1.0 + dist)
            a[dist == 0] = 0.0
            al[k] = a.astype(np.float32)
            k += 1
    _ALPHA_CACHE[q] = al
    return al


def _prep_host(X):
    """Per-core rotated input dicts. X: [B, L, D] float32."""
    in_maps = []
    for c in range(N_CORES):
        b, q = c // 4, c % 4
        rot = 128 * q
        Xr = np.roll(X[b], -rot, axis=0)          # program token t = real t+rot
        xtT = np.ascontiguousarray(Xr.T)          # [D, L]
        xt16 = xtT.astype(ml_dtypes.bfloat16)
        xt32b = np.ascontiguousarray(
            np.concatenate([xtT[:, 0:128], xtT[:, 512:640]], axis=1))
        xaug = np.concatenate(
            [Xr, np.ones((L, 1), dtype=np.float32)], axis=1)
        xaug = np.ascontiguousarray(xaug.reshape(NBLK, 128, D + 1))
        xaug16 = xaug.astype(ml_dtypes.bfloat16)
        in_maps.append({"xt16": xt16, "xt32b": xt32b, "xaug": xaug,
                        "xaug16": xaug16, "alphat": _core_alpha(q)})
    return in_maps


def _get_compiled():
    global _COMPILED
    if _COMPILED is None:
        _COMPILED = _build()
    return _COMPILED


def kernel(X, _trace=False, _trace_kwargs=None):
    """X: np.ndarray [2, 1024, 128] float32 -> O [2, 1024, 128] float32."""
    from concourse.bass_utils import run_bass_kernel_spmd

    X = np.asarray(X, dtype=np.float32)
    assert X.shape == (B, L, D)
    nc = _get_compiled()
    in_maps = _prep_host(X)
    res = run_bass_kernel_spmd(nc, in_maps, list(range(N_CORES)),
                               trace=_trace, **(_trace_kwargs or {}))
    O = np.zeros((B, L, D), dtype=np.float32)
    for c in range(N_CORES):
        b, q = c // 4, c % 4
        part = res.results[c]["pout"].reshape(L, D)
        O[b] += np.roll(part, 128 * q, axis=0)    # un-rotate
    if _trace:
        return O, res
    return O


if __name__ == "__main__":
    rng = np.random.default_rng(0)
    X = rng.standard_normal((B, L, D), dtype=np.float32)
    O = kernel(X)
    print("ok", O.shape, float(np.abs(O).max()))
